# revision 15
# baseline (speedup 1.0000x reference)
"""Positional embedding lookup kernel for Trainium2 (8 NeuronCores).

Problem: out[b, t, :] = tok_weight[x[b, t], :] + pos_weight[t, :]
  x:          [4, 4096]  int32/int64 token ids in [0, 32000)
  tok_weight: [32000, 512] f32
  pos_weight: [4096, 512]  f32
  out:        [4, 4096, 512] f32

Sharding: split the 4096 positions into 8 contiguous chunks of 512; core c
handles positions [c*512, (c+1)*512) for ALL 4 batches (2048 tokens).  This
makes each core read only its 1MB slice of pos_weight (reused across the 4
batches) instead of a per-token 4MB read.

Per-core flat token order: i = 0..2047 walks (b, q) = (i//512, i%512),
i.e. flat_idx = x[:, c*512:(c+1)*512].ravel().  The gather lands token i at
SBUF partition i%128, column-block i//128, so column block col corresponds
to batch col//4, position sub-block col%4 — which aligns a whole batch's
512 tokens with the (identically laid out) pos tile for a single wide add.

The row gather uses the GPSIMD dma_gather custom op (one descriptor per
row, ~0.34ns/descriptor generation) in 4 chunks of 512 rows so gather,
add, and store pipeline; indices are int16 (vocab 32000 < 32768), packed
i -> [i%16, i//16] over 16 partitions and replicated across the 8 Q7 cores.
"""

import numpy as np

import concourse.bass as bass
import concourse.tile as tile
from concourse import library_config, mybir
from concourse.bass_utils import run_bass_kernel_spmd

B = 4
T = 4096
E = 512
VOCAB = 32000
N_CORES = 8
POS_PER_CORE = T // N_CORES          # 512
TOK_PER_CORE = B * POS_PER_CORE      # 2048
P = 128
N_TILES = TOK_PER_CORE // P          # 16 column blocks of 128 tokens
JQ = POS_PER_CORE // P               # 4 pos sub-blocks
CHUNKS = 4                           # one gather/add/store chunk per batch
TOK_PER_CHUNK = TOK_PER_CORE // CHUNKS   # 512
IDX_COLS = TOK_PER_CORE // 16        # 128 int16 idx columns

_CACHE = {}


def _split_multi_waits(nc: bass.Bass) -> None:
    """Walrus codegen allows one sync-wait slot per TPB instruction (the
    NEURON_ISA_TPB_EVENTS struct); Tile can emit several.  Move extra waits
    onto standalone NoOps on the same engine, just before the instruction."""
    for func in nc.m.functions:
        for blk in func.blocks:
            new_insts = []
            for inst in blk.instructions:
                si = inst.sync_info
                if si is not None and len(si.on_wait) > 1:
                    for w in si.on_wait[:-1]:
                        nop = mybir.InstNoOp(
                            name=nc.get_next_instruction_name(),
                            engine=inst.engine,
                            bass_nofuse=True,
                            sync_info=mybir.SyncInfo(on_wait=[w], on_update=[]),
                        )
                        nc.register_instruction(nop)
                        new_insts.append(nop)
                    inst.sync_info = mybir.SyncInfo(
                        on_wait=si.on_wait[-1:], on_update=si.on_update
                    )
                new_insts.append(inst)
            blk.instructions[:] = new_insts


def _build_program(reps: int = 1, outer: int = 1) -> bass.Bass:
    """reps>1 unrolls the steady-state gather/add/store loop; outer>1 wraps
    it in a runtime For_i loop.  Used for timing: the wall-time delta
    between two total rep counts isolates device time."""
    nc = bass.Bass()

    xti = nc.declare_dram_parameter(
        "xti", [P, IDX_COLS], mybir.dt.int16, isOutput=False
    )
    pos = nc.declare_dram_parameter(
        "pos", [POS_PER_CORE, E], mybir.dt.float32, isOutput=False
    )
    tok = nc.declare_dram_parameter(
        "tok", [VOCAB, E], mybir.dt.float32, isOutput=False
    )
    out = nc.declare_dram_parameter(
        "out", [N_TILES, P, E], mybir.dt.float32, isOutput=True
    )

    with tile.TileContext(nc) as tc:
        with (
            tc.tile_pool(name="const", bufs=1) as const_pool,
            tc.tile_pool(name="work", bufs=3) as work_pool,
        ):
            # dma_gather lives in the 'mlp' GPSIMD firmware library
            nc.gpsimd.load_library(library_config.mlp)

            xti_t = const_pool.tile([P, IDX_COLS], mybir.dt.int16)
            nc.sync.dma_start(out=xti_t[:], in_=xti[:])

            # all 4 pos sub-blocks in one DMA: partition p, cols
            # [jq*E:(jq+1)*E] hold pos[jq*128 + p, :]
            pos_t = const_pool.tile([P, JQ * E], mybir.dt.float32)
            nc.sync.dma_start(
                out=pos_t[:].rearrange("p (jq e) -> p jq e", jq=JQ),
                in_=pos.rearrange("(jq p) e -> p jq e", p=P),
            )
            # tiny DVE op so the vector engine observes the const-load DMA
            # semaphores once; later adds then need only the gather wait.
            obs = const_pool.tile([P, 1], mybir.dt.float32, tag="obs")
            nc.vector.tensor_copy(out=obs[:], in_=pos_t[:, 0:1])

            ncols = N_TILES // CHUNKS            # 4 column blocks per chunk
            icols = IDX_COLS // CHUNKS           # 32 idx columns per chunk
            nidx_reg = nc.gpsimd.to_reg(TOK_PER_CHUNK)

            def body():
                for _ in range(reps):
                    for k in range(CHUNKS):
                        g = work_pool.tile(
                            [P, ncols * E], mybir.dt.float32, tag="work"
                        )
                        nc.gpsimd.dma_gather(
                            g[:].rearrange("p (c e) -> p c e", e=E),
                            tok[:],
                            xti_t[:, k * icols : (k + 1) * icols],
                            TOK_PER_CHUNK,
                            nidx_reg,
                            E,
                        )
                        nc.vector.tensor_add(out=g[:], in0=g[:], in1=pos_t[:])
                        nc.sync.dma_start(
                            out=out[k * ncols : (k + 1) * ncols].rearrange(
                                "c p e -> p c e"
                            ),
                            in_=g[:].rearrange("p (c e) -> p c e", e=E),
                        )

            if outer > 1:
                with tc.For_i(0, outer):
                    body()
            else:
                body()

    # populate .instr bytes for extended-inst InstISA subclasses (the
    # library-reload pseudo); Bacc runs this in compile(), raw Bass doesn't
    from concourse.library_overlay import lower_extended_insts

    lower_extended_insts(nc)
    _split_multi_waits(nc)
    return nc


def make_in_maps(x32: np.ndarray, tokw: np.ndarray, posw: np.ndarray):
    in_maps = []
    for c in range(N_CORES):
        flat = x32[:, c * POS_PER_CORE : (c + 1) * POS_PER_CORE].reshape(-1)
        flat16 = flat.astype(np.int16)
        # idx i -> [i%16, i//16], replicated across the 8 groups of 16
        # partitions (one replica per GPSIMD Q7 core)
        wrapped = flat16.reshape(IDX_COLS, 16).T          # [16, 128]
        xti = np.ascontiguousarray(np.tile(wrapped, (8, 1)))  # [128, 128]
        pc = np.ascontiguousarray(posw[c * POS_PER_CORE : (c + 1) * POS_PER_CORE])
        in_maps.append({"xti": xti, "pos": pc, "tok": tokw})
    return in_maps


def unshard(results) -> np.ndarray:
    full = np.empty((B, T, E), dtype=np.float32)
    for c in range(N_CORES):
        oc = results[c]["out"]  # [16, 128, 512]; token i at [i//128, i%128]
        full[:, c * POS_PER_CORE : (c + 1) * POS_PER_CORE, :] = oc.reshape(
            B, JQ, P, E
        ).reshape(B, POS_PER_CORE, E)
    return full


def kernel(x: np.ndarray, tok_weight: np.ndarray, pos_weight: np.ndarray) -> np.ndarray:
    if "nc" not in _CACHE:
        _CACHE["nc"] = _build_program()
    nc = _CACHE["nc"]

    x32 = np.ascontiguousarray(np.asarray(x, dtype=np.int32))
    tokw = np.ascontiguousarray(np.asarray(tok_weight, dtype=np.float32))
    posw = np.ascontiguousarray(np.asarray(pos_weight, dtype=np.float32))

    in_maps = make_in_maps(x32, tokw, posw)
    results = run_bass_kernel_spmd(nc, in_maps, core_ids=list(range(N_CORES))).results
    return unshard(results)


# revision 17
# speedup vs baseline: 2.8416x; 2.8416x over previous
"""Positional embedding lookup kernel for Trainium2 (8 NeuronCores).

Problem: out[b, t, :] = tok_weight[x[b, t], :] + pos_weight[t, :]
  x:          [4, 4096]  int32/int64 token ids in [0, 32000)
  tok_weight: [32000, 512] f32
  pos_weight: [4096, 512]  f32
  out:        [4, 4096, 512] f32

Sharding: split the 4096 positions into 8 contiguous chunks of 512; core c
handles positions [c*512, (c+1)*512) for ALL 4 batches (2048 tokens).  This
makes each core read only its 1MB slice of pos_weight (reused across the 4
batches) instead of a per-token 4MB read.

Per-core flat token order: i = 0..2047 walks (b, q) = (i//512, i%512),
i.e. flat_idx = x[:, c*512:(c+1)*512].ravel().  The gather lands token i at
SBUF partition i%128, column-block i//128, so column block col corresponds
to batch col//4, position sub-block col%4 — which aligns a whole batch's
512 tokens with the (identically laid out) pos tile for a single wide add.

The row gather uses the GPSIMD dma_gather custom op (one descriptor per
row, ~0.34ns/descriptor generation) in 4 chunks of 512 rows so gather,
add, and store pipeline; indices are int16 (vocab 32000 < 32768), packed
i -> [i%16, i//16] over 16 partitions and replicated across the 8 Q7 cores.
"""

import numpy as np

import concourse.bass as bass
import concourse.tile as tile
from concourse import library_config, mybir
from concourse.bass_utils import run_bass_kernel_spmd

B = 4
T = 4096
E = 512
VOCAB = 32000
N_CORES = 8
POS_PER_CORE = T // N_CORES          # 512
TOK_PER_CORE = B * POS_PER_CORE      # 2048
P = 128
N_TILES = TOK_PER_CORE // P          # 16 column blocks of 128 tokens
JQ = POS_PER_CORE // P               # 4 pos sub-blocks
CHUNKS = 4                           # one gather/add/store chunk per batch
TOK_PER_CHUNK = TOK_PER_CORE // CHUNKS   # 512
IDX_COLS = TOK_PER_CORE // 16        # 128 int16 idx columns

_CACHE = {}


def _split_multi_waits(nc: bass.Bass) -> None:
    """Walrus codegen allows one sync-wait slot per TPB instruction (the
    NEURON_ISA_TPB_EVENTS struct); Tile can emit several.  Move extra waits
    onto standalone NoOps on the same engine, just before the instruction."""
    for func in nc.m.functions:
        for blk in func.blocks:
            new_insts = []
            for inst in blk.instructions:
                si = inst.sync_info
                if si is not None and len(si.on_wait) > 1:
                    for w in si.on_wait[:-1]:
                        nop = mybir.InstNoOp(
                            name=nc.get_next_instruction_name(),
                            engine=inst.engine,
                            bass_nofuse=True,
                            sync_info=mybir.SyncInfo(on_wait=[w], on_update=[]),
                        )
                        nc.register_instruction(nop)
                        new_insts.append(nop)
                    inst.sync_info = mybir.SyncInfo(
                        on_wait=si.on_wait[-1:], on_update=si.on_update
                    )
                new_insts.append(inst)
            blk.instructions[:] = new_insts


def _build_program(reps: int = 1, outer: int = 1, variant: str = "full") -> bass.Bass:
    """reps>1 unrolls the steady-state gather/add/store loop; outer>1 wraps
    it in a runtime For_i loop.  Used for timing: the wall-time delta
    between two total rep counts isolates device time.  variant isolates
    pipeline stages for benching: "full" | "gather" | "store"."""
    nc = bass.Bass()

    xti = nc.declare_dram_parameter(
        "xti", [P, IDX_COLS], mybir.dt.int16, isOutput=False
    )
    pos = nc.declare_dram_parameter(
        "pos", [POS_PER_CORE, E], mybir.dt.float32, isOutput=False
    )
    tok = nc.declare_dram_parameter(
        "tok", [VOCAB, E], mybir.dt.float32, isOutput=False
    )
    out = nc.declare_dram_parameter(
        "out", [N_TILES, P, E], mybir.dt.float32, isOutput=True
    )

    with tile.TileContext(nc) as tc:
        with (
            tc.tile_pool(name="const", bufs=1) as const_pool,
            tc.tile_pool(name="work", bufs=3) as work_pool,
        ):
            # dma_gather lives in the 'mlp' GPSIMD firmware library
            nc.gpsimd.load_library(library_config.mlp)

            xti_t = const_pool.tile([P, IDX_COLS], mybir.dt.int16)
            nc.sync.dma_start(out=xti_t[:], in_=xti[:])

            # all 4 pos sub-blocks in one DMA: partition p, cols
            # [jq*E:(jq+1)*E] hold pos[jq*128 + p, :]
            pos_t = const_pool.tile([P, JQ * E], mybir.dt.float32)
            nc.sync.dma_start(
                out=pos_t[:].rearrange("p (jq e) -> p jq e", jq=JQ),
                in_=pos.rearrange("(jq p) e -> p jq e", p=P),
            )
            # tiny DVE op so the vector engine observes the const-load DMA
            # semaphores once; later adds then need only the gather wait.
            obs = const_pool.tile([P, 1], mybir.dt.float32, tag="obs")
            nc.vector.tensor_copy(out=obs[:], in_=pos_t[:, 0:1])

            ncols = N_TILES // CHUNKS            # 4 column blocks per chunk
            icols = IDX_COLS // CHUNKS           # 32 idx columns per chunk
            nidx_reg = nc.gpsimd.to_reg(TOK_PER_CHUNK)

            g0 = None
            if variant == "store":
                g0 = const_pool.tile([P, ncols * E], mybir.dt.float32, tag="g0")
                nc.gpsimd.dma_gather(
                    g0[:].rearrange("p (c e) -> p c e", e=E),
                    tok[:],
                    xti_t[:, 0:icols],
                    TOK_PER_CHUNK,
                    nidx_reg,
                    E,
                )
                nc.vector.tensor_add(out=g0[:], in0=g0[:], in1=pos_t[:])

            def body():
                for _ in range(reps):
                    for k in range(CHUNKS):
                        if variant == "store":
                            g = g0
                        else:
                            g = work_pool.tile(
                                [P, ncols * E], mybir.dt.float32, tag="work"
                            )
                            nc.gpsimd.dma_gather(
                                g[:].rearrange("p (c e) -> p c e", e=E),
                                tok[:],
                                xti_t[:, k * icols : (k + 1) * icols],
                                TOK_PER_CHUNK,
                                nidx_reg,
                                E,
                            )
                        if variant == "full":
                            nc.vector.tensor_add(out=g[:], in0=g[:], in1=pos_t[:])
                        if variant in ("full", "store"):
                            nc.sync.dma_start(
                                out=out[k * ncols : (k + 1) * ncols].rearrange(
                                    "c p e -> p c e"
                                ),
                                in_=g[:].rearrange("p (c e) -> p c e", e=E),
                            )

            if outer > 1:
                with tc.For_i(0, outer):
                    body()
            else:
                body()

    # populate .instr bytes for extended-inst InstISA subclasses (the
    # library-reload pseudo); Bacc runs this in compile(), raw Bass doesn't
    from concourse.library_overlay import lower_extended_insts

    lower_extended_insts(nc)
    _split_multi_waits(nc)
    return nc


def make_in_maps(x32: np.ndarray, tokw: np.ndarray, posw: np.ndarray):
    in_maps = []
    for c in range(N_CORES):
        flat = x32[:, c * POS_PER_CORE : (c + 1) * POS_PER_CORE].reshape(-1)
        flat16 = flat.astype(np.int16)
        # idx i -> [i%16, i//16], replicated across the 8 groups of 16
        # partitions (one replica per GPSIMD Q7 core)
        wrapped = flat16.reshape(IDX_COLS, 16).T          # [16, 128]
        xti = np.ascontiguousarray(np.tile(wrapped, (8, 1)))  # [128, 128]
        pc = np.ascontiguousarray(posw[c * POS_PER_CORE : (c + 1) * POS_PER_CORE])
        in_maps.append({"xti": xti, "pos": pc, "tok": tokw})
    return in_maps


def unshard(results) -> np.ndarray:
    full = np.empty((B, T, E), dtype=np.float32)
    for c in range(N_CORES):
        oc = results[c]["out"]  # [16, 128, 512]; token i at [i//128, i%128]
        full[:, c * POS_PER_CORE : (c + 1) * POS_PER_CORE, :] = oc.reshape(
            B, JQ, P, E
        ).reshape(B, POS_PER_CORE, E)
    return full


def kernel(x: np.ndarray, tok_weight: np.ndarray, pos_weight: np.ndarray) -> np.ndarray:
    if "nc" not in _CACHE:
        _CACHE["nc"] = _build_program()
    nc = _CACHE["nc"]

    x32 = np.ascontiguousarray(np.asarray(x, dtype=np.int32))
    tokw = np.ascontiguousarray(np.asarray(tok_weight, dtype=np.float32))
    posw = np.ascontiguousarray(np.asarray(pos_weight, dtype=np.float32))

    in_maps = make_in_maps(x32, tokw, posw)
    results = run_bass_kernel_spmd(nc, in_maps, core_ids=list(range(N_CORES))).results
    return unshard(results)
